# revision 63
# baseline (speedup 1.0000x reference)
"""Trainium2 Bass kernel for the 1x1-conv attention module.

Shapes (hardcoded): x (8, 64, 64, 64) fp32, w_qkv (192, 64), b_qkv (192,),
w_out (64, 64), b_out (64,). Data-parallel: one batch element per NeuronCore
(8 cores). Everything stays channel-major (c on partitions, t = h*64+w on the
free dim); the reference's view/permute "quirk" composes to the standard
channel-major permute, so no data movement is needed for it.

Per-core pipeline (measured ~173 us/core on TRN2, rel err 1.3e-4):
  qkv projections via TensorE in float32r (K=65 rows: a ones row of x
  absorbs the biases; softmax scale folded into w_q host-side).  q and k
  are duplicated into both SBUF partition halves (SBUF->SBUF DMA) so the
  QK^T matmuls row-pack two K=64 j-tiles into the 128x128 PE array.
  Scores are computed transposed, s_T[j, i] = k_j . q_i, so the exp()
  output already has j on partitions and feeds the PV matmul directly as
  the moving operand.  v is computed token-major with an appended ones
  column (M=65) so the PV accumulation also produces the softmax row-sums
  for free.  exp() runs on ScalarE straight out of PSUM in 3-bank (1536
  elem) groups; no max subtraction (scores are bounded ~|8|, exact fp32).
  Emission is software-pipelined for the in-order engine queues: wave w's
  PV matmuls are emitted after wave w+1's QK so the PE overlaps QK with
  exp, and the deferred-PV pipeline carries ACROSS i-chunk boundaries so
  the next chunk's first exp is never gated behind the previous chunk's
  final PV matmuls; the PV accumulator is double-buffered and drains to
  SBUF off the critical path; all normalization (reciprocal + K=1
  broadcast matmul + multiply) and the output projection + residual run
  as a batched tail so no PE instruction mid-loop ever waits on the
  Vector engine.
"""

import numpy as np

B, C, HW = 8, 64, 4096
NCORES = 8
IC = 512  # i-chunk (query tokens per block)
NIC = HW // IC  # 8
NJ = HW // 128  # 32 j-chunks of 128 tokens

_compiled = None


def _build_bass(repeat=1, do_exp=True, do_av=True, do_norm=True,
                av_fixed_tile=False, pipelined=True, exp_frac=None,
                use_bf16=False, exp_preload=True, epool_bufs=3,
                stat_bf16=False, split_pools=False, u_bufs=2,
                overlap_ends=False, stream_proj=False):
    import concourse.bass as bass
    import concourse.mybir as mybir
    import concourse.tile as tile

    FP = mybir.dt.float32
    FR = mybir.dt.float32r
    FB = mybir.dt.bfloat16
    DT = FB if use_bf16 else FR
    ST = FB if (stat_bf16 or use_bf16) else DT  # stationary dtype
    ICW = IC  # bf16 N=1024 impossible: matmul out must be fp32 (1 bank)
    NICW = HW // ICW
    Exp = mybir.ActivationFunctionType.Exp

    nc = bass.Bass("TRN2", target_bir_lowering=False, debug=False)

    xa_d = nc.dram_tensor("xa", [C + 1, HW], FP, kind="ExternalInput")
    xb_d = nc.dram_tensor("xb", [C, HW], FP, kind="ExternalInput")
    wq_d = nc.dram_tensor("wq", [C + 1, C], FP, kind="ExternalInput")
    wk_d = nc.dram_tensor("wk", [C + 1, C], FP, kind="ExternalInput")
    wv_d = nc.dram_tensor("wv", [C + 1, C], FP, kind="ExternalInput")
    wo_d = nc.dram_tensor("wo", [C, C], FP, kind="ExternalInput")
    out_d = nc.dram_tensor("out", [C, HW], FP, kind="ExternalOutput")

    with tile.TileContext(nc) as tc:
        with (
            nc.allow_low_precision(reason="fp32r matmul operands (full fp32 accum in PSUM)"),
            tc.tile_pool(name="singles", bufs=1) as singles,
            tc.tile_pool(name="escr", bufs=epool_bufs) as epool,
            tc.tile_pool(name="att", bufs=2) as apool,
            tc.tile_pool(name="outp", bufs=2) as opool,
            tc.tile_pool(name="small", bufs=2) as smallp,
            tc.tile_pool(name="usb", bufs=2) as uspool,
            tc.tile_pool(name="sps", bufs=2, space="PSUM") as spool,
            tc.tile_pool(name="psA", bufs=1, space="PSUM") as poolA,
            tc.tile_pool(name="psB", bufs=1, space="PSUM") as poolB,
            tc.tile_pool(name="ups", bufs=u_bufs, space="PSUM") as upool,
            tc.tile_pool(name="rps", bufs=1, space="PSUM") as rpool,
        ):
            # ---- load inputs ----
            xa = singles.tile([C + 1, HW], FP)
            xb = singles.tile([C, HW], FP)
            wq = singles.tile([C + 1, C], FP)
            wk = singles.tile([C + 1, C], FP)
            wv = singles.tile([C + 1, C], FP)
            wo = singles.tile([C, C], FP)
            nc.sync.dma_start(out=xa[:], in_=xa_d[:])
            nc.sync.dma_start(out=xb[:], in_=xb_d[:])
            nc.sync.dma_start(out=wq[:], in_=wq_d[:])
            nc.sync.dma_start(out=wk[:], in_=wk_d[:])
            nc.sync.dma_start(out=wv[:], in_=wv_d[:])
            nc.sync.dma_start(out=wo[:], in_=wo_d[:])

            # fp32r copies of the DMA-loaded matmul operands (walrus requires
            # matmul inputs to be produced rounded-to-fp32r by an engine op)
            xar = singles.tile([C + 1, HW], FR)
            wqr = singles.tile([C + 1, C], FR)
            wkr = singles.tile([C + 1, C], FR)
            wvr = singles.tile([C + 1, C], FR)
            wor = singles.tile([C, C], FR)
            nc.vector.tensor_copy(xar[:], xa[:])
            nc.vector.tensor_copy(wqr[:], wq[:])
            nc.vector.tensor_copy(wkr[:], wk[:])
            nc.vector.tensor_copy(wvr[:], wv[:])
            nc.vector.tensor_copy(wor[:], wo[:])

            ones32 = singles.tile([128, 1], FP)
            nc.vector.memset(ones32[:], 1.0)
            if exp_preload:
                # preload the exp table set while DMAs are in flight
                expwarm = singles.tile([1, 1], FP)
                nc.scalar.activation(expwarm[:], ones32[0:1, :], Exp)
            ones_b = singles.tile([1, C], FR)  # K=1 stationary for broadcast
            nc.vector.tensor_copy(
                ones_b[:], ones32[0:1, 0:1].to_broadcast([1, C])
            )
            # unnormalized PV output + rowsums, all i-chunks (drained here
            # so the PSUM accumulator bank frees up each i-chunk)
            u_all = singles.tile([C + 1, NIC, IC], FP)
            r_all = singles.tile([1, HW], FR)
            att_all = singles.tile([C, HW], FR)  # normalized attention

            # ---- projections ----
            # q, k: channel-major (64, HW), duplicated into both partition
            # halves so QK can row-pack pairs of j-tiles.
            qd = singles.tile([128, HW], DT)
            kd = singles.tile([128, HW], ST)
            # v token-major tiles: vt[:, jc, 0:64] = v rows for j-chunk jc,
            # vt[:, jc, 64] = 1.0 (row-sum column).
            vt = singles.tile([128, NJ, C + 1], ST)
            for jc in range(NJ):
                nc.vector.tensor_copy(vt[:, jc, C : C + 1], ones32[:])

            # ---- attention main loop ----
            if split_pools:
                # alternate 4-bank / 3-bank slots: fewer, wider exp reads
                # while the two single-buffered pools still double-buffer
                # each other
                wave_sizes = [4, 3, 4, 3, 4, 3, 4, 3, 4]
            else:
                wave_sizes = []
                left = NJ
                while left > 0:
                    w = min(3, left)
                    wave_sizes.append(w)
                    left -= w
            assert sum(wave_sizes) == NJ

            _slot_flip = [0]
            def wave_slot(ws=None):
                if not split_pools:
                    return spool.tile([128, 3, IC], FP, tag="scores", name="sw")
                if ws is None:
                    _slot_flip[0] ^= 1
                    ws = 4 if _slot_flip[0] else 3
                if ws == 4:
                    return poolA.tile([128, 4, IC], FP, tag="wavesA", name="swA")
                return poolB.tile([128, 3, IC], FP, tag="wavesB", name="swB")

            def emit_kq_chunk(dst, w_, n, via_u=False):
                sl = slice(n * IC, (n + 1) * IC)
                if via_u:
                    psu = upool.tile([C + 1, IC], FP, tag="u", name="prju")
                    pso = psu[0:C, :]
                elif overlap_ends:
                    ps = rpool.tile([C, IC], FP, tag="rsmall", name="prj")
                    pso = ps[:]
                else:
                    ps3 = wave_slot()
                    pso = ps3[0:C, 0, :]
                nc.tensor.matmul(pso, w_[:], xar[:, sl], start=True, stop=True)
                nc.vector.tensor_copy(dst[0:C, sl], pso)
                # duplicate into partitions 64..127 for QK row-packing
                nc.sync.dma_start(out=dst[C:128, sl], in_=dst[0:C, sl])

            def emit_v_group(g, via_u=False):
                # batch 8 token-chunks per PSUM bank so the chain doesn't
                # serialize MM-copy-MM-copy through one slot
                if via_u:
                    pvu = upool.tile([128, 8, C], FP, tag="u", name="pvu")
                    pv = pvu[:]
                elif overlap_ends:
                    pv = rpool.tile([128, 8, C], FP, tag="rsmall", name="pv")
                elif use_bf16 or split_pools or u_bufs > 1:
                    pvs = wave_slot()
                    pv = pvs[:, 0, :].rearrange("p (a b) -> p a b", a=8)
                else:
                    pv = rpool.tile([128, 8, C], FP, tag="rsmall", name="pv")
                for t in range(8):
                    jc = g * 8 + t
                    jsl = slice(jc * 128, (jc + 1) * 128)
                    nc.tensor.matmul(
                        pv[:, t, :],
                        xar[:, jsl],
                        wvr[:],
                        start=True,
                        stop=True,
                    )
                nc.vector.tensor_copy(
                    vt[:, g * 8 : (g + 1) * 8, 0:C], pv[:]
                )

            def emit_norm_chunk(ic):
                # rb = broadcast(1/rowsum); att = U * rb  (PSUM via rpool)
                isl = slice(ic * IC, (ic + 1) * IC)
                rb = rpool.tile([C, IC], FP, tag="rsmall", name="rb")
                nc.tensor.matmul(
                    rb[:], ones_b[:], r_all[:, isl], start=True, stop=True
                )
                nc.vector.tensor_mul(
                    att_all[:, isl], u_all[0:C, ic, :], rb[:]
                )

            def emit_out_chunk(ic):
                isl = slice(ic * IC, (ic + 1) * IC)
                p = rpool.tile([C, IC], FP, tag="rsmall", name="po")
                nc.tensor.matmul(
                    p[:], wor[:], att_all[:, isl], start=True, stop=True
                )
                o = opool.tile([C, IC], FP, name="o")
                nc.vector.tensor_add(o[:], p[:], xb[:, isl])
                nc.sync.dma_start(out=out_d[:, isl], in_=o[:])

            for _rep in range(repeat):
              if stream_proj:
                # upfront: only what the first waves of i-chunk 0 need;
                # the rest streams through the idle U slot, one item per
                # wave, scheduled by need-by (global wave index).
                emit_kq_chunk(kd, wkr, 0)
                emit_kq_chunk(kd, wkr, 1)
                emit_kq_chunk(qd, wqr, 0)
                emit_v_group(0)
                emit_v_group(1)
                stream_items = [
                    (("k", 2), 0), (("k", 3), 1), (("v", 2), 2),
                    (("k", 4), 3), (("k", 5), 4), (("v", 3), 5),
                    (("k", 6), 6), (("k", 7), 7), (("q", 1), 8),
                    (("q", 2), 9), (("q", 3), 10), (("q", 4), 14),
                    (("q", 5), 15), (("q", 6), 16), (("q", 7), 17),
                ]
                pending_work = []
              elif overlap_ends:
                # upfront: only what waves 0-2 of i-chunk 0 need
                emit_kq_chunk(kd, wkr, 0)
                emit_kq_chunk(kd, wkr, 1)
                emit_kq_chunk(qd, wqr, 0)
                emit_v_group(0)
                pending_work = (
                    [("k", n) for n in range(2, NIC)]
                    + [("v", g) for g in range(1, NJ // 8)]
                    + [("q", n) for n in range(1, NIC)]
                )
                stream_items = []
              else:
                for dst, w_ in ((qd, wqr), (kd, wkr)):
                    for n in range(NIC):
                        emit_kq_chunk(dst, w_, n)
                for g in range(NJ // 8):
                    emit_v_group(g)
                pending_work = []
                stream_items = []

              NH = ICW // 512  # PSUM-bank halves per wave row
              def emit_av(u, e3, ws, jbase):
                for t in range(ws):
                    jj = jbase + t
                    for h in range(NH):
                        hs = slice(h * 512, (h + 1) * 512)
                        nc.tensor.matmul(
                            u[:, hs],
                            vt[:, 0 if av_fixed_tile else jj, :],
                            e3[:, t, hs],
                            start=(jj == 0),
                            stop=(jj == NJ - 1),
                        )



              def emit_drain(pic, pu):
                # drain u to SBUF to free its PSUM bank, and kick off the
                # reciprocal (DVE-only; no PE stall)
                csl = slice(pic * NH, (pic + 1) * NH)
                nc.vector.tensor_copy(
                    u_all[:, csl, :],
                    pu[:].rearrange("p (a b) -> p a b", a=NH),
                )
                if do_norm:
                    nc.vector.reciprocal(
                        r_all[:, pic * ICW : (pic + 1) * ICW],
                        u_all[C : C + 1, csl, :],
                    )

              # deferred-AV pipeline carried ACROSS i-chunk boundaries so
              # the next chunk's first QK (and thus its exp) is never
              # gated behind the previous chunk's final PV matmuls.
              prev = None  # (ic, u, e3, ws, jbase) awaiting AV
              for ic in range(NICW):
                isl = slice(ic * ICW, (ic + 1) * ICW)
                u = upool.tile([C + 1, ICW], FP, tag="u")
                jc = 0
                for w, ws in enumerate(wave_sizes):
                    # feed deferred start-up projections into the early
                    # wave stream (they pace through the spare PSUM bank)
                    gw = ic * len(wave_sizes) + w
                    if stream_items and stream_items[0][1] <= gw:
                        (kind, arg), _sched = stream_items.pop(0)
                        if kind == "k":
                            emit_kq_chunk(kd, wkr, arg, via_u=True)
                        elif kind == "q":
                            emit_kq_chunk(qd, wqr, arg, via_u=True)
                        else:
                            emit_v_group(arg, via_u=True)
                    for _ in range(2):
                        if pending_work:
                            kind, arg = pending_work.pop(0)
                            if kind == "k":
                                emit_kq_chunk(kd, wkr, arg)
                            elif kind == "q":
                                emit_kq_chunk(qd, wqr, arg)
                            else:
                                emit_v_group(arg)
                    # lagged normalization/output of the previous i-chunk
                    if overlap_ends and pipelined and do_norm and ic >= 1:
                        if w == 3:
                            emit_norm_chunk(ic - 1)
                        elif w == 5:
                            emit_out_chunk(ic - 1)
                    s3 = wave_slot(ws)
                    e3 = epool.tile([128, 4 if split_pools else 3, ICW], DT)
                    jbase = jc
                    for t in range(ws):
                        half = jc % 2
                        hsl = slice(64 * half, 64 * (half + 1))
                        nc.tensor.matmul(
                            s3[:, t, :],
                            kd[hsl, jc * 128 : (jc + 1) * 128],
                            qd[hsl, isl],
                            start=True,
                            stop=True,
                        )
                        jc += 1
                    if do_exp:
                        we = min(ws, exp_frac) if exp_frac else ws
                        nc.scalar.activation(
                            e3[:, 0:we, :], s3[:, 0:we, :], Exp
                        )
                    if pipelined:
                        if prev is not None and do_av:
                            pic, pu, pe3, pws, pjbase = prev
                            emit_av(pu, pe3, pws, pjbase)
                            if pic != ic:
                                emit_drain(pic, pu)
                        prev = (ic, u, e3, ws, jbase)
                    elif do_av:
                        emit_av(u, e3, ws, jbase)
                        if w == len(wave_sizes) - 1:
                            emit_drain(ic, u)
              if pipelined and prev is not None and do_av:
                pic, pu, pe3, pws, pjbase = prev
                emit_av(pu, pe3, pws, pjbase)
                emit_drain(pic, pu)
              if overlap_ends and pipelined and do_norm:
                emit_norm_chunk(NICW - 1)
                emit_out_chunk(NICW - 1)

              # ---- tail: normalize + output projection + residual ----
              if do_norm and not (overlap_ends and pipelined):
                for ic in range(NIC):
                    isl = slice(ic * IC, (ic + 1) * IC)
                    rb3 = wave_slot(4 if split_pools else None)
                    nc.tensor.matmul(
                        rb3[0:C, 0, :],
                        ones_b[:],
                        r_all[:, isl],
                        start=True,
                        stop=True,
                    )
                    nc.vector.tensor_mul(
                        att_all[:, isl], u_all[0:C, ic, :], rb3[0:C, 0, :]
                    )
                for ic in range(NIC):
                    isl = slice(ic * IC, (ic + 1) * IC)
                    p3 = wave_slot(3 if split_pools else None)
                    nc.tensor.matmul(
                        p3[0:C, 0, :],
                        wor[:],
                        att_all[:, isl],
                        start=True,
                        stop=True,
                    )
                    o = opool.tile([C, IC], FP)
                    nc.vector.tensor_add(o[:], p3[0:C, 0, :], xb[:, isl])
                    nc.sync.dma_start(out=out_d[:, isl], in_=o[:])

    _split_matmul_waits(nc, mybir)
    return nc


def _split_matmul_waits(nc, mybir):
    """walrus's codegen only has room for one sync-wait in the engine
    micro-op structs; peel extra waits off onto wait-only EventSemaphore
    instructions on the same engine queue just before.

    First, drop waits that are trivially satisfied: a sem-ge wait on a
    semaphore that is only ever incremented by instructions on this same
    (in-order, FIFO-completing) engine queue is redundant -- by the time
    this instruction dispatches, all its predecessors have completed."""
    skip = (mybir.InstEventSemaphore,)
    # map sem id -> set of engines that update it (and whether any update
    # is something other than a plain increment)
    sem_engines = {}
    sem_clean = {}
    for bb in nc.main_func.blocks:
        for ins in bb.instructions:
            si = ins.sync_info
            if si is None or not si.on_update:
                continue
            for up in si.on_update:
                sem_engines.setdefault(up.id, set()).add(str(ins.engine))
                # DMA sem increments fire at (async) DMA completion, not
                # at queue progress -- never treat those as queue-ordered
                ok = (
                    up.update_mode == "sem-inc"
                    and up.update_reg is None
                    and "DMA" not in type(ins).__name__
                )
                sem_clean[up.id] = sem_clean.get(up.id, True) and ok

    def is_redundant(ins, wait):
        return (
            wait.wait_mode == "sem-ge-imm"
            and wait.wait_reg is None
            and sem_clean.get(wait.id, False)
            and sem_engines.get(wait.id) == {str(ins.engine)}
        )

    for bb in nc.main_func.blocks:
        for ins in bb.instructions:
            if isinstance(ins, skip):
                continue
            si = ins.sync_info
            if si is not None and si.on_wait and len(si.on_wait) > 1:
                kept = [w for w in si.on_wait if not is_redundant(ins, w)]
                if len(kept) != len(si.on_wait):
                    if not kept:
                        kept = [si.on_wait[-1]]
                    ins.sync_info = mybir.SyncInfo(
                        on_wait=kept, on_update=list(si.on_update or [])
                    )
    for bb in nc.main_func.blocks:
        insts = list(bb.instructions)
        out = []
        changed = False
        for ins in insts:
            if not isinstance(ins, skip):
                si = ins.sync_info
                if si is not None and si.on_wait and len(si.on_wait) > 1:
                    for wi, wait in enumerate(si.on_wait[:-1]):
                        w = mybir.InstEventSemaphore(
                            name=f"{ins.name}_prewait{wi}"
                        )
                        w.engine = ins.engine
                        w.sync_info = mybir.SyncInfo(
                            on_wait=[wait], on_update=[]
                        )
                        out.append(w)
                    ins.sync_info = mybir.SyncInfo(
                        on_wait=[si.on_wait[-1]],
                        on_update=list(si.on_update or []),
                    )
                    changed = True
            out.append(ins)
        if changed:
            bb.instructions = out


def _prep_inputs(x, w_qkv, b_qkv, w_out, b_out):
    """Host-side input prep -> per-core in_maps."""
    x = np.ascontiguousarray(np.asarray(x, dtype=np.float32))
    w_qkv = np.asarray(w_qkv, dtype=np.float32)
    b_qkv = np.asarray(b_qkv, dtype=np.float32)
    w_out = np.asarray(w_out, dtype=np.float32)
    b_out = np.asarray(b_out, dtype=np.float32)

    scale = 1.0 / np.sqrt(np.float32(C))
    wq = np.concatenate([w_qkv[0:C].T, b_qkv[None, 0:C]], axis=0) * scale
    wk = np.concatenate([w_qkv[C : 2 * C].T, b_qkv[None, C : 2 * C]], axis=0)
    wv = np.concatenate([w_qkv[2 * C :].T, b_qkv[None, 2 * C :]], axis=0)
    wo = np.ascontiguousarray(w_out.T)
    wq = np.ascontiguousarray(wq, dtype=np.float32)
    wk = np.ascontiguousarray(wk, dtype=np.float32)
    wv = np.ascontiguousarray(wv, dtype=np.float32)

    ones = np.ones((1, HW), dtype=np.float32)
    in_maps = []
    for b in range(B):
        xcm = x[b].reshape(C, HW)
        xa = np.concatenate([xcm, ones], axis=0)
        xb = xcm + b_out[:, None].astype(np.float32)
        in_maps.append(
            {
                "xa": np.ascontiguousarray(xa),
                "xb": np.ascontiguousarray(xb),
                "wq": wq,
                "wk": wk,
                "wv": wv,
                "wo": wo,
            }
        )
    return in_maps


def _get_compiled():
    global _compiled
    if _compiled is None:
        _compiled = _build_bass()
    return _compiled


def kernel(x, w_qkv, b_qkv, w_out, b_out, _trace=False, _trace_kwargs=None):
    from concourse.bass_utils import run_bass_kernel_spmd

    nc = _get_compiled()
    in_maps = _prep_inputs(x, w_qkv, b_qkv, w_out, b_out)
    res = run_bass_kernel_spmd(
        nc,
        in_maps,
        list(range(NCORES)),
        trace=_trace,
        **(_trace_kwargs or {}),
    )
    out = np.stack([res.results[b]["out"].reshape(C, 64, 64) for b in range(B)])
    if _trace:
        kernel._last_results = res
    return out.astype(np.float32)


# revision 65
# speedup vs baseline: 1.0139x; 1.0139x over previous
"""Trainium2 Bass kernel for the 1x1-conv attention module.

Shapes (hardcoded): x (8, 64, 64, 64) fp32, w_qkv (192, 64), b_qkv (192,),
w_out (64, 64), b_out (64,). Data-parallel: one batch element per NeuronCore
(8 cores). Everything stays channel-major (c on partitions, t = h*64+w on the
free dim); the reference's view/permute "quirk" composes to the standard
channel-major permute, so no data movement is needed for it.

Per-core pipeline (measured ~173 us/core on TRN2, rel err 1.3e-4):
  qkv projections via TensorE in float32r (K=65 rows: a ones row of x
  absorbs the biases; softmax scale folded into w_q host-side).  q and k
  are duplicated into both SBUF partition halves (SBUF->SBUF DMA) so the
  QK^T matmuls row-pack two K=64 j-tiles into the 128x128 PE array.
  Scores are computed transposed, s_T[j, i] = k_j . q_i, so the exp()
  output already has j on partitions and feeds the PV matmul directly as
  the moving operand.  v is computed token-major with an appended ones
  column (M=65) so the PV accumulation also produces the softmax row-sums
  for free.  exp() runs on ScalarE straight out of PSUM in 3-bank (1536
  elem) groups; no max subtraction (scores are bounded ~|8|, exact fp32).
  Emission is software-pipelined for the in-order engine queues: wave w's
  PV matmuls are emitted after wave w+1's QK so the PE overlaps QK with
  exp, and the deferred-PV pipeline carries ACROSS i-chunk boundaries so
  the next chunk's first exp is never gated behind the previous chunk's
  final PV matmuls; the PV accumulator is double-buffered and drains to
  SBUF off the critical path; all normalization (reciprocal + K=1
  broadcast matmul + multiply) and the output projection + residual run
  as a batched tail so no PE instruction mid-loop ever waits on the
  Vector engine.
"""

import numpy as np

B, C, HW = 8, 64, 4096
NCORES = 8
IC = 512  # i-chunk (query tokens per block)
NIC = HW // IC  # 8
NJ = HW // 128  # 32 j-chunks of 128 tokens

_compiled = None


def _build_bass(repeat=1, do_exp=True, do_av=True, do_norm=True,
                av_fixed_tile=False, pipelined=True, exp_frac=None,
                use_bf16=False, exp_preload=True, epool_bufs=5,
                stat_bf16=False, split_pools=False, u_bufs=2,
                overlap_ends=False, stream_proj=False, pair_waves=True,
                ):
    import concourse.bass as bass
    import concourse.mybir as mybir
    import concourse.tile as tile

    FP = mybir.dt.float32
    FR = mybir.dt.float32r
    FB = mybir.dt.bfloat16
    DT = FB if use_bf16 else FR
    ST = FB if (stat_bf16 or use_bf16) else DT  # stationary dtype
    ICW = IC  # bf16 N=1024 impossible: matmul out must be fp32 (1 bank)
    NICW = HW // ICW
    Exp = mybir.ActivationFunctionType.Exp

    nc = bass.Bass("TRN2", target_bir_lowering=False, debug=False)

    xa_d = nc.dram_tensor("xa", [C + 1, HW], FP, kind="ExternalInput")
    xb_d = nc.dram_tensor("xb", [C, HW], FP, kind="ExternalInput")
    wq_d = nc.dram_tensor("wq", [C + 1, C], FP, kind="ExternalInput")
    wk_d = nc.dram_tensor("wk", [C + 1, C], FP, kind="ExternalInput")
    wv_d = nc.dram_tensor("wv", [C + 1, C], FP, kind="ExternalInput")
    wo_d = nc.dram_tensor("wo", [C, C], FP, kind="ExternalInput")
    out_d = nc.dram_tensor("out", [C, HW], FP, kind="ExternalOutput")

    with tile.TileContext(nc) as tc:
        with (
            nc.allow_low_precision(reason="fp32r matmul operands (full fp32 accum in PSUM)"),
            tc.tile_pool(name="singles", bufs=1) as singles,
            tc.tile_pool(name="escr", bufs=epool_bufs) as epool,
            tc.tile_pool(name="att", bufs=2) as apool,
            tc.tile_pool(name="outp", bufs=2) as opool,
            tc.tile_pool(name="small", bufs=2) as smallp,
            tc.tile_pool(name="usb", bufs=2) as uspool,
            tc.tile_pool(name="sps", bufs=2, space="PSUM") as spool,
            tc.tile_pool(name="psA", bufs=1, space="PSUM") as poolA,
            tc.tile_pool(name="psB", bufs=1, space="PSUM") as poolB,
            tc.tile_pool(name="ups", bufs=u_bufs, space="PSUM") as upool,
            tc.tile_pool(name="rps", bufs=1, space="PSUM") as rpool,
        ):
            # ---- load inputs ----
            xa = singles.tile([C + 1, HW], FP)
            xb = singles.tile([C, HW], FP)
            wq = singles.tile([C + 1, C], FP)
            wk = singles.tile([C + 1, C], FP)
            wv = singles.tile([C + 1, C], FP)
            wo = singles.tile([C, C], FP)
            nc.sync.dma_start(out=xa[:], in_=xa_d[:])
            nc.sync.dma_start(out=xb[:], in_=xb_d[:])
            nc.sync.dma_start(out=wq[:], in_=wq_d[:])
            nc.sync.dma_start(out=wk[:], in_=wk_d[:])
            nc.sync.dma_start(out=wv[:], in_=wv_d[:])
            nc.sync.dma_start(out=wo[:], in_=wo_d[:])

            # fp32r copies of the DMA-loaded matmul operands (walrus requires
            # matmul inputs to be produced rounded-to-fp32r by an engine op)
            xar = singles.tile([C + 1, HW], FR)
            wqr = singles.tile([C + 1, C], FR)
            wkr = singles.tile([C + 1, C], FR)
            wvr = singles.tile([C + 1, C], FR)
            wor = singles.tile([C, C], FR)
            nc.vector.tensor_copy(xar[:], xa[:])
            nc.vector.tensor_copy(wqr[:], wq[:])
            nc.vector.tensor_copy(wkr[:], wk[:])
            nc.vector.tensor_copy(wvr[:], wv[:])
            nc.vector.tensor_copy(wor[:], wo[:])

            ones32 = singles.tile([128, 1], FP)
            nc.vector.memset(ones32[:], 1.0)
            if exp_preload:
                # preload the exp table set while DMAs are in flight
                expwarm = singles.tile([1, 1], FP)
                nc.scalar.activation(expwarm[:], ones32[0:1, :], Exp)
            ones_b = singles.tile([1, C], FR)  # K=1 stationary for broadcast
            nc.vector.tensor_copy(
                ones_b[:], ones32[0:1, 0:1].to_broadcast([1, C])
            )
            # unnormalized PV output + rowsums, all i-chunks (drained here
            # so the PSUM accumulator bank frees up each i-chunk)
            u_all = singles.tile([C + 1, NIC, IC], FP)
            r_all = singles.tile([1, HW], FR)
            att_all = singles.tile([C, HW], FR)  # normalized attention

            # ---- projections ----
            # q, k: channel-major (64, HW), duplicated into both partition
            # halves so QK can row-pack pairs of j-tiles.
            qd = singles.tile([128, HW], DT)
            kd = singles.tile([128, HW], ST)
            # v token-major tiles: vt[:, jc, 0:64] = v rows for j-chunk jc,
            # vt[:, jc, 64] = 1.0 (row-sum column).
            vt = singles.tile([128, NJ, C + 1], ST)
            for jc in range(NJ):
                nc.vector.tensor_copy(vt[:, jc, C : C + 1], ones32[:])

            # ---- attention main loop ----
            if split_pools:
                # alternate 4-bank / 3-bank slots: fewer, wider exp reads
                # while the two single-buffered pools still double-buffer
                # each other
                wave_sizes = [4, 3, 4, 3, 4, 3, 4, 3, 4]
            else:
                wave_sizes = []
                left = NJ
                while left > 0:
                    w = min(3, left)
                    wave_sizes.append(w)
                    left -= w
            assert sum(wave_sizes) == NJ

            _slot_flip = [0]
            def wave_slot(ws=None):
                if not split_pools:
                    return spool.tile([128, 3, IC], FP, tag="scores", name="sw")
                if ws is None:
                    _slot_flip[0] ^= 1
                    ws = 4 if _slot_flip[0] else 3
                if ws == 4:
                    return poolA.tile([128, 4, IC], FP, tag="wavesA", name="swA")
                return poolB.tile([128, 3, IC], FP, tag="wavesB", name="swB")

            def emit_kq_chunk(dst, w_, n, via_u=False):
                sl = slice(n * IC, (n + 1) * IC)
                if via_u:
                    psu = upool.tile([C + 1, IC], FP, tag="u", name="prju")
                    pso = psu[0:C, :]
                elif overlap_ends:
                    ps = rpool.tile([C, IC], FP, tag="rsmall", name="prj")
                    pso = ps[:]
                else:
                    ps3 = wave_slot()
                    pso = ps3[0:C, 0, :]
                nc.tensor.matmul(pso, w_[:], xar[:, sl], start=True, stop=True)
                nc.vector.tensor_copy(dst[0:C, sl], pso)
                # duplicate into partitions 64..127 for QK row-packing
                nc.sync.dma_start(out=dst[C:128, sl], in_=dst[0:C, sl])

            def emit_v_group(g, via_u=False):
                # batch 8 token-chunks per PSUM bank so the chain doesn't
                # serialize MM-copy-MM-copy through one slot
                if via_u:
                    pvu = upool.tile([128, 8, C], FP, tag="u", name="pvu")
                    pv = pvu[:]
                elif overlap_ends:
                    pv = rpool.tile([128, 8, C], FP, tag="rsmall", name="pv")
                elif use_bf16 or split_pools or u_bufs > 1:
                    pvs = wave_slot()
                    pv = pvs[:, 0, :].rearrange("p (a b) -> p a b", a=8)
                else:
                    pv = rpool.tile([128, 8, C], FP, tag="rsmall", name="pv")
                for t in range(8):
                    jc = g * 8 + t
                    jsl = slice(jc * 128, (jc + 1) * 128)
                    nc.tensor.matmul(
                        pv[:, t, :],
                        xar[:, jsl],
                        wvr[:],
                        start=True,
                        stop=True,
                    )
                nc.vector.tensor_copy(
                    vt[:, g * 8 : (g + 1) * 8, 0:C], pv[:]
                )

            def emit_norm_chunk(ic):
                # rb = broadcast(1/rowsum); att = U * rb  (PSUM via rpool)
                isl = slice(ic * IC, (ic + 1) * IC)
                rb = rpool.tile([C, IC], FP, tag="rsmall", name="rb")
                nc.tensor.matmul(
                    rb[:], ones_b[:], r_all[:, isl], start=True, stop=True
                )
                nc.vector.tensor_mul(
                    att_all[:, isl], u_all[0:C, ic, :], rb[:]
                )

            def emit_out_chunk(ic):
                isl = slice(ic * IC, (ic + 1) * IC)
                p = rpool.tile([C, IC], FP, tag="rsmall", name="po")
                nc.tensor.matmul(
                    p[:], wor[:], att_all[:, isl], start=True, stop=True
                )
                o = opool.tile([C, IC], FP, name="o")
                nc.vector.tensor_add(o[:], p[:], xb[:, isl])
                nc.sync.dma_start(out=out_d[:, isl], in_=o[:])

            for _rep in range(repeat):
              if stream_proj:
                # upfront: only what the first waves of i-chunk 0 need;
                # the rest streams through the idle U slot, one item per
                # wave, scheduled by need-by (global wave index).
                emit_kq_chunk(kd, wkr, 0)
                emit_kq_chunk(kd, wkr, 1)
                emit_kq_chunk(qd, wqr, 0)
                emit_v_group(0)
                emit_v_group(1)
                stream_items = [
                    (("k", 2), 0), (("k", 3), 1), (("v", 2), 2),
                    (("k", 4), 3), (("k", 5), 4), (("v", 3), 5),
                    (("k", 6), 6), (("k", 7), 7), (("q", 1), 8),
                    (("q", 2), 9), (("q", 3), 10), (("q", 4), 14),
                    (("q", 5), 15), (("q", 6), 16), (("q", 7), 17),
                ]
                pending_work = []
              elif overlap_ends:
                # upfront: only what waves 0-2 of i-chunk 0 need
                emit_kq_chunk(kd, wkr, 0)
                emit_kq_chunk(kd, wkr, 1)
                emit_kq_chunk(qd, wqr, 0)
                emit_v_group(0)
                pending_work = (
                    [("k", n) for n in range(2, NIC)]
                    + [("v", g) for g in range(1, NJ // 8)]
                    + [("q", n) for n in range(1, NIC)]
                )
                stream_items = []
              else:
                for dst, w_ in ((qd, wqr), (kd, wkr)):
                    for n in range(NIC):
                        emit_kq_chunk(dst, w_, n)
                for g in range(NJ // 8):
                    emit_v_group(g)
                pending_work = []
                stream_items = []

              NH = ICW // 512  # PSUM-bank halves per wave row
              def emit_av(u, e3, ws, jbase):
                for t in range(ws):
                    jj = jbase + t
                    for h in range(NH):
                        hs = slice(h * 512, (h + 1) * 512)
                        nc.tensor.matmul(
                            u[:, hs],
                            vt[:, 0 if av_fixed_tile else jj, :],
                            e3[:, t, hs],
                            start=(jj == 0),
                            stop=(jj == NJ - 1),
                        )



              def emit_drain(pic, pu):
                # drain u to SBUF to free its PSUM bank, and kick off the
                # reciprocal (DVE-only; no PE stall)
                csl = slice(pic * NH, (pic + 1) * NH)
                nc.vector.tensor_copy(
                    u_all[:, csl, :],
                    pu[:].rearrange("p (a b) -> p a b", a=NH),
                )
                if do_norm:
                    nc.vector.reciprocal(
                        r_all[:, pic * ICW : (pic + 1) * ICW],
                        u_all[C : C + 1, csl, :],
                    )

              # deferred-AV pipeline carried ACROSS i-chunk boundaries so
              # the next chunk's first QK (and thus its exp) is never
              # gated behind the previous chunk's final PV matmuls.
              if pair_waves and pipelined:
                # ---- wave-pair blocks: QK for two waves back-to-back
                # (complete row-pack pairing), both exps queued, previous
                # block's PV matmuls after -- PV fully decoupled from the
                # exp chain.
                blocks_per_ic = []
                i = 0
                while i < len(wave_sizes):
                    blocks_per_ic.append(wave_sizes[i : i + 2])
                    i += 2
                prev_block = []  # (ic, u, e3, ws, jbase) awaiting AV
                for ic in range(NICW):
                    isl = slice(ic * ICW, (ic + 1) * ICW)
                    u = upool.tile([C + 1, ICW], FP, tag="u")
                    jc = 0
                    for block in blocks_per_ic:
                        cur = []
                        exps = []
                        for ws in block:
                            s3 = wave_slot(ws)
                            e3 = epool.tile(
                                [128, 4 if split_pools else 3, ICW], DT
                            )
                            jbase = jc
                            for t in range(ws):
                                half = jc % 2
                                hsl = slice(64 * half, 64 * (half + 1))
                                nc.tensor.matmul(
                                    s3[:, t, :],
                                    kd[hsl, jc * 128 : (jc + 1) * 128],
                                    qd[hsl, isl],
                                    start=True,
                                    stop=True,
                                )
                                jc += 1
                            exps.append((e3, s3, ws))
                            cur.append((ic, u, e3, ws, jbase))
                        if do_exp:
                            for e3x, s3x, wsx in exps:
                                nc.scalar.activation(
                                    e3x[:, 0:wsx, :], s3x[:, 0:wsx, :], Exp
                                )
                        if do_av:
                            for pic, pu, pe3, pws, pjbase in prev_block:
                                emit_av(pu, pe3, pws, pjbase)
                                if pjbase + pws == NJ:
                                    emit_drain(pic, pu)
                        prev_block = cur
                if do_av:
                    for pic, pu, pe3, pws, pjbase in prev_block:
                        emit_av(pu, pe3, pws, pjbase)
                        if pjbase + pws == NJ:
                            emit_drain(pic, pu)
                prev = None
              else:
               prev = None  # (ic, u, e3, ws, jbase) awaiting AV
               for ic in range(NICW):
                isl = slice(ic * ICW, (ic + 1) * ICW)
                u = upool.tile([C + 1, ICW], FP, tag="u")
                jc = 0
                for w, ws in enumerate(wave_sizes):
                    # feed deferred start-up projections into the early
                    # wave stream (they pace through the spare PSUM bank)
                    gw = ic * len(wave_sizes) + w
                    if stream_items and stream_items[0][1] <= gw:
                        (kind, arg), _sched = stream_items.pop(0)
                        if kind == "k":
                            emit_kq_chunk(kd, wkr, arg, via_u=True)
                        elif kind == "q":
                            emit_kq_chunk(qd, wqr, arg, via_u=True)
                        else:
                            emit_v_group(arg, via_u=True)
                    for _ in range(2):
                        if pending_work:
                            kind, arg = pending_work.pop(0)
                            if kind == "k":
                                emit_kq_chunk(kd, wkr, arg)
                            elif kind == "q":
                                emit_kq_chunk(qd, wqr, arg)
                            else:
                                emit_v_group(arg)
                    # lagged normalization/output of the previous i-chunk
                    if overlap_ends and pipelined and do_norm and ic >= 1:
                        if w == 3:
                            emit_norm_chunk(ic - 1)
                        elif w == 5:
                            emit_out_chunk(ic - 1)
                    s3 = wave_slot(ws)
                    e3 = epool.tile([128, 4 if split_pools else 3, ICW], DT)
                    jbase = jc
                    for t in range(ws):
                        half = jc % 2
                        hsl = slice(64 * half, 64 * (half + 1))
                        nc.tensor.matmul(
                            s3[:, t, :],
                            kd[hsl, jc * 128 : (jc + 1) * 128],
                            qd[hsl, isl],
                            start=True,
                            stop=True,
                        )
                        jc += 1
                    if do_exp:
                        we = min(ws, exp_frac) if exp_frac else ws
                        nc.scalar.activation(
                            e3[:, 0:we, :], s3[:, 0:we, :], Exp
                        )
                    if pipelined:
                        if prev is not None and do_av:
                            pic, pu, pe3, pws, pjbase = prev
                            emit_av(pu, pe3, pws, pjbase)
                            if pic != ic:
                                emit_drain(pic, pu)
                        prev = (ic, u, e3, ws, jbase)
                    elif do_av:
                        emit_av(u, e3, ws, jbase)
                        if w == len(wave_sizes) - 1:
                            emit_drain(ic, u)
              if pipelined and prev is not None and do_av:
                pic, pu, pe3, pws, pjbase = prev
                emit_av(pu, pe3, pws, pjbase)
                emit_drain(pic, pu)
              if overlap_ends and pipelined and do_norm:
                emit_norm_chunk(NICW - 1)
                emit_out_chunk(NICW - 1)

              # ---- tail: normalize + output projection + residual ----
              if do_norm and not (overlap_ends and pipelined):
                for ic in range(NIC):
                    isl = slice(ic * IC, (ic + 1) * IC)
                    rb3 = wave_slot(4 if split_pools else None)
                    nc.tensor.matmul(
                        rb3[0:C, 0, :],
                        ones_b[:],
                        r_all[:, isl],
                        start=True,
                        stop=True,
                    )
                    nc.vector.tensor_mul(
                        att_all[:, isl], u_all[0:C, ic, :], rb3[0:C, 0, :]
                    )
                for ic in range(NIC):
                    isl = slice(ic * IC, (ic + 1) * IC)
                    p3 = wave_slot(3 if split_pools else None)
                    nc.tensor.matmul(
                        p3[0:C, 0, :],
                        wor[:],
                        att_all[:, isl],
                        start=True,
                        stop=True,
                    )
                    o = opool.tile([C, IC], FP)
                    nc.vector.tensor_add(o[:], p3[0:C, 0, :], xb[:, isl])
                    nc.sync.dma_start(out=out_d[:, isl], in_=o[:])

    _split_matmul_waits(nc, mybir)
    return nc


def _split_matmul_waits(nc, mybir):
    """walrus's codegen only has room for one sync-wait in the engine
    micro-op structs; peel extra waits off onto wait-only EventSemaphore
    instructions on the same engine queue just before.

    First, drop waits that are trivially satisfied: a sem-ge wait on a
    semaphore that is only ever incremented by instructions on this same
    (in-order, FIFO-completing) engine queue is redundant -- by the time
    this instruction dispatches, all its predecessors have completed."""
    skip = (mybir.InstEventSemaphore,)
    # map sem id -> set of engines that update it (and whether any update
    # is something other than a plain increment)
    sem_engines = {}
    sem_clean = {}
    for bb in nc.main_func.blocks:
        for ins in bb.instructions:
            si = ins.sync_info
            if si is None or not si.on_update:
                continue
            for up in si.on_update:
                sem_engines.setdefault(up.id, set()).add(str(ins.engine))
                # DMA sem increments fire at (async) DMA completion, not
                # at queue progress -- never treat those as queue-ordered
                ok = (
                    up.update_mode == "sem-inc"
                    and up.update_reg is None
                    and "DMA" not in type(ins).__name__
                )
                sem_clean[up.id] = sem_clean.get(up.id, True) and ok

    def is_redundant(ins, wait):
        return (
            wait.wait_mode == "sem-ge-imm"
            and wait.wait_reg is None
            and sem_clean.get(wait.id, False)
            and sem_engines.get(wait.id) == {str(ins.engine)}
        )

    for bb in nc.main_func.blocks:
        for ins in bb.instructions:
            if isinstance(ins, skip):
                continue
            si = ins.sync_info
            if si is not None and si.on_wait and len(si.on_wait) > 1:
                kept = [w for w in si.on_wait if not is_redundant(ins, w)]
                if len(kept) != len(si.on_wait):
                    if not kept:
                        kept = [si.on_wait[-1]]
                    ins.sync_info = mybir.SyncInfo(
                        on_wait=kept, on_update=list(si.on_update or [])
                    )
    for bb in nc.main_func.blocks:
        insts = list(bb.instructions)
        out = []
        changed = False
        for ins in insts:
            if not isinstance(ins, skip):
                si = ins.sync_info
                if si is not None and si.on_wait and len(si.on_wait) > 1:
                    for wi, wait in enumerate(si.on_wait[:-1]):
                        w = mybir.InstEventSemaphore(
                            name=f"{ins.name}_prewait{wi}"
                        )
                        w.engine = ins.engine
                        w.sync_info = mybir.SyncInfo(
                            on_wait=[wait], on_update=[]
                        )
                        out.append(w)
                    ins.sync_info = mybir.SyncInfo(
                        on_wait=[si.on_wait[-1]],
                        on_update=list(si.on_update or []),
                    )
                    changed = True
            out.append(ins)
        if changed:
            bb.instructions = out


def _prep_inputs(x, w_qkv, b_qkv, w_out, b_out):
    """Host-side input prep -> per-core in_maps."""
    x = np.ascontiguousarray(np.asarray(x, dtype=np.float32))
    w_qkv = np.asarray(w_qkv, dtype=np.float32)
    b_qkv = np.asarray(b_qkv, dtype=np.float32)
    w_out = np.asarray(w_out, dtype=np.float32)
    b_out = np.asarray(b_out, dtype=np.float32)

    scale = 1.0 / np.sqrt(np.float32(C))
    wq = np.concatenate([w_qkv[0:C].T, b_qkv[None, 0:C]], axis=0) * scale
    wk = np.concatenate([w_qkv[C : 2 * C].T, b_qkv[None, C : 2 * C]], axis=0)
    wv = np.concatenate([w_qkv[2 * C :].T, b_qkv[None, 2 * C :]], axis=0)
    wo = np.ascontiguousarray(w_out.T)
    wq = np.ascontiguousarray(wq, dtype=np.float32)
    wk = np.ascontiguousarray(wk, dtype=np.float32)
    wv = np.ascontiguousarray(wv, dtype=np.float32)

    ones = np.ones((1, HW), dtype=np.float32)
    in_maps = []
    for b in range(B):
        xcm = x[b].reshape(C, HW)
        xa = np.concatenate([xcm, ones], axis=0)
        xb = xcm + b_out[:, None].astype(np.float32)
        in_maps.append(
            {
                "xa": np.ascontiguousarray(xa),
                "xb": np.ascontiguousarray(xb),
                "wq": wq,
                "wk": wk,
                "wv": wv,
                "wo": wo,
            }
        )
    return in_maps


def _get_compiled():
    global _compiled
    if _compiled is None:
        _compiled = _build_bass()
    return _compiled


def kernel(x, w_qkv, b_qkv, w_out, b_out, _trace=False, _trace_kwargs=None):
    from concourse.bass_utils import run_bass_kernel_spmd

    nc = _get_compiled()
    in_maps = _prep_inputs(x, w_qkv, b_qkv, w_out, b_out)
    res = run_bass_kernel_spmd(
        nc,
        in_maps,
        list(range(NCORES)),
        trace=_trace,
        **(_trace_kwargs or {}),
    )
    out = np.stack([res.results[b]["out"].reshape(C, 64, 64) for b in range(B)])
    if _trace:
        kernel._last_results = res
    return out.astype(np.float32)
